# revision 1
# baseline (speedup 1.0000x reference)
"""DiagonalAttention Trainium2 kernel (Bass/Tile), data-parallel over batch.

Reference computation per batch b (L=2048, D=H=1024):
    r1 = relu(x1 @ W.T) * diag        [L, H]   (diag>0 folded into W1)
    r2 = relu(x2 @ W.T)               [L, H]
    s  = r1 @ r2.T + (1-mask)*NEG     [L, L]
    out = softmax(s, -1) @ x2         [L, D]

Device strategy per core (B_LOC=2 batches):
  - host: transpose x1/x2 to [D, L] fp16, W/W1 -> WT fp16, x2 bf16 copy for
    the output matmul, mask row bf16.
  - proj (fp16 matmuls, fp32 psum): rT[h, l] accumulated over d-chunks,
    relu on ScalarE -> fp16.
  - scores (fp16): psum[i=128, j=2048]; mask row added via K=1 bf16 starter
    matmuls; ScalarE copies scores to SBUF, VectorE row-max, ScalarE
    exp(s-max) -> bf16 E with fused row-sum (accum_out).
  - E transposed 128x128 on the PE (bf16); bmm3 = ET.T @ x2_bf16 in psum,
    scaled by 1/z on VectorE during psum->SBUF copy, DMA out.
  - PE stream software-pipelined one i-chunk: transposes+bmm3 of chunk i
    are emitted after the scores matmuls of chunk i+1, so the PE never
    stalls on the softmax chain.

This container's walrus allows ONE sync wait per instruction; the
legalization pass below splits multi-wait instructions (absorber DMAs on
the same ring for DMAs — HWDGE rings evaluate descriptor waits in FIFO
order — and NoOps on the same engine otherwise).
"""
import copy
import numpy as np

B, L, D, H = 16, 2048, 1024, 1024
NCORES = 8
B_LOC = B // NCORES
NEG = -10000.0

ND = D // 128    # d chunks (contraction of proj)
NH = H // 128    # h chunks
NI = L // 128    # i chunks per batch
SW = 512         # proj slab width (moving dim)
NS = L // SW     # slabs per batch
IPS = SW // 128  # i-chunks per slab
JW = 512         # scores moving width
NJ = L // JW     # j chunks in scores
NJ3 = L // 128   # j chunks in bmm3 (stationary ET tiles)


# ---------------------------------------------------------------------------
# Wait-count legalization


def _make_wait_scratch(nc):
    """Scratch DRAM + token DMA (call inside the TileContext); the token
    gives a fully-lowered physical-AP DMACopy to clone as absorber."""
    from concourse import mybir

    src = nc.dram_tensor("legal_src", [1, 16], mybir.dt.float32, kind="Internal")
    dst = nc.dram_tensor("legal_dst", [1, 16], mybir.dt.float32, kind="Internal")
    tok = nc.sync.dma_start(dst.ap()[0:1, 0:1], src.ap()[0:1, 0:1])
    return tok.ins


def _legalize_waits(nc, template_inst, max_waits=1):
    """Split every instruction with more than max_waits sync waits.

    DMACopy: insert tiny absorber DMAs on the same queue (ring-FIFO makes
    the carried waits gate the real DMA).  Engine instructions: insert
    NoOps on the same engine (engines dispatch strictly in order; a
    waiting NoOp stalls everything behind it).
    """
    from concourse import mybir

    sem = nc.alloc_semaphore("legal_junk")
    junk = mybir.SyncUpdate(
        sync_type="semaphore", id=getattr(sem, "num", None),
        update_mode="sem-add-imm", update_value=16,
        ant_name=getattr(sem, "name", "legal_junk"))
    for fn in nc.m.functions:
        for blk in fn.blocks:
            out = []
            for inst in blk.instructions:
                si = getattr(inst, "sync_info", None)
                if si is None or si.on_wait is None or len(si.on_wait) <= max_waits:
                    out.append(inst)
                    continue
                waits = list(si.on_wait)
                extra, keep = waits[:-max_waits], waits[-max_waits:]
                if isinstance(inst, mybir.InstDMACopy):
                    for w in extra:
                        ab = copy.deepcopy(template_inst)
                        ab.name = nc.get_next_instruction_name()
                        ab.queue = inst.queue
                        ab.sync_info = mybir.SyncInfo(
                            on_wait=[w], on_update=[copy.deepcopy(junk)])
                        out.append(ab)
                else:
                    for w in extra:
                        out.append(mybir.InstNoOp(
                            name=nc.get_next_instruction_name(),
                            engine=inst.engine,
                            sync_info=mybir.SyncInfo(on_wait=[w], on_update=[])))
                inst.sync_info = mybir.SyncInfo(
                    on_wait=keep, on_update=list(si.on_update or []))
                out.append(inst)
            blk.instructions[:] = out


# ---------------------------------------------------------------------------
# Program


def _build_program(reps=None, use_mask=True):
    """Build the program.  reps=k wraps the whole computation in a
    hardware For loop executing it k times — used only for marginal
    HW timing (the ~100 ms axon dispatch swamps a single ~1 ms exec).
    use_mask=False elides the K=1 mask-starter matmuls (the mask input
    must then be all-ones so its additive row is exactly zero)."""
    import concourse.bass as bass
    import concourse.tile as tile
    from concourse import mybir
    from concourse.masks import make_identity

    dt = mybir.dt
    nc = bass.Bass("TRN2", target_bir_lowering=False, debug=False)

    x1T = nc.dram_tensor("x1T", [B_LOC, D, L], dt.float16, kind="ExternalInput").ap()
    x2T = nc.dram_tensor("x2T", [B_LOC, D, L], dt.float16, kind="ExternalInput").ap()
    WT = nc.dram_tensor("WT", [D, H], dt.float16, kind="ExternalInput").ap()
    W1T = nc.dram_tensor("W1T", [D, H], dt.float16, kind="ExternalInput").ap()
    x2n = nc.dram_tensor("x2n", [B_LOC, L, D], dt.bfloat16, kind="ExternalInput").ap()
    mrow = nc.dram_tensor("mrow", [B_LOC, 1, L], dt.bfloat16, kind="ExternalInput").ap()
    out = nc.dram_tensor("out", [B_LOC, L, D], dt.float32, kind="ExternalOutput").ap()

    with tile.TileContext(nc) as tc:
        with (
            tc.tile_pool(name="const", bufs=1) as cpool,
            tc.tile_pool(name="big", bufs=1) as bigp,
            tc.tile_pool(name="slab", bufs=2) as slabp,
            tc.tile_pool(name="r1pool", bufs=2) as r1p,
            tc.tile_pool(name="work", bufs=1) as workp,
            tc.tile_pool(name="tep", bufs=2) as tep,
            tc.tile_pool(name="small", bufs=2) as smallp,
            tc.tile_pool(name="outp", bufs=4) as outp,
            tc.tile_pool(name="ps_s", bufs=1, space="PSUM") as ps_s,
            tc.tile_pool(name="ps_p", bufs=2, space="PSUM") as ps_p,
            tc.tile_pool(name="ps_sm", bufs=2, space="PSUM") as ps_sm,
        ):
            tok = _make_wait_scratch(nc)

            identbf = cpool.tile([128, 128], dt.bfloat16, tag="identbf")
            make_identity(nc, identbf[:])
            ones_bf = cpool.tile([1, 128], dt.bfloat16, tag="ones_bf")
            nc.vector.memset(ones_bf[:], 1.0)

            # resident weights
            wt = bigp.tile([128, ND, H], dt.float16, tag="wt")
            w1t = bigp.tile([128, ND, H], dt.float16, tag="w1t")
            for dc in range(ND):
                nc.sync.dma_start(
                    wt[:, dc, :], WT.rearrange("(c p) h -> p c h", p=128)[:, dc, :])
            for dc in range(ND):
                nc.sync.dma_start(
                    w1t[:, dc, :], W1T.rearrange("(c p) h -> p c h", p=128)[:, dc, :])

            # per-batch resident tensors
            r2T = bigp.tile([128, NH, L], dt.float16, tag="r2T")
            t_x2n = bigp.tile([128, NJ3, D], dt.bfloat16, tag="t_x2n")
            t_m = workp.tile([1, L], dt.bfloat16, tag="t_m")
            sco = workp.tile([128, L], dt.float32, tag="sco")
            tET = workp.tile([128, NJ3, 128], dt.bfloat16, tag="tET")

            def emit_all_batches():
              pending = [None]
              for b in range(B_LOC):
                if pending[0] is not None:
                    pending[0]()
                    pending[0] = None
                # ---- batch loads ----
                for jc in range(NJ3):
                    nc.sync.dma_start(
                        t_x2n[:, jc, :],
                        x2n[b].rearrange("(c p) d -> p c d", p=128)[:, jc, :])
                nc.sync.dma_start(t_m[:], mrow[b])

                # ---- proj2: r2T = relu(W @ x2T) ----
                for s in range(NS):
                    xs = slabp.tile([128, ND, SW], dt.float16, tag="xslab")
                    for dc in range(ND):
                        nc.sync.dma_start(
                            xs[:, dc, :],
                            x2T[b].rearrange("(c p) l -> p c l", p=128)[
                                :, dc, s * SW:(s + 1) * SW])
                    for hc in range(NH):
                        psp = ps_p.tile([128, SW], dt.float32, tag="psp")
                        for dc in range(ND):
                            nc.tensor.matmul(
                                psp[:], wt[:, dc, hc * 128:(hc + 1) * 128],
                                xs[:, dc, :],
                                start=(dc == 0), stop=(dc == ND - 1))
                        nc.scalar.activation(
                            r2T[:, hc, s * SW:(s + 1) * SW], psp[:],
                            mybir.ActivationFunctionType.Relu)

                # ---- proj1 + attention, slab by slab ----
                for s in range(NS):
                    xs = slabp.tile([128, ND, SW], dt.float16, tag="xslab")
                    for dc in range(ND):
                        nc.sync.dma_start(
                            xs[:, dc, :],
                            x1T[b].rearrange("(c p) l -> p c l", p=128)[
                                :, dc, s * SW:(s + 1) * SW])
                    r1s = r1p.tile([128, NH, SW], dt.float16, tag="r1slab")
                    for hc in range(NH):
                        psp = ps_p.tile([128, SW], dt.float32, tag="psp")
                        for dc in range(ND):
                            nc.tensor.matmul(
                                psp[:], w1t[:, dc, hc * 128:(hc + 1) * 128],
                                xs[:, dc, :],
                                start=(dc == 0), stop=(dc == ND - 1))
                        nc.scalar.activation(
                            r1s[:, hc, :], psp[:],
                            mybir.ActivationFunctionType.Relu)

                    for il in range(IPS):
                        ic = s * IPS + il
                        isl = slice(il * 128, (il + 1) * 128)
                        pss = ps_s.tile([128, L], dt.float32, tag="pss")
                        if use_mask:
                            for jc in range(NJ):
                                nc.tensor.matmul(
                                    pss[:, jc * JW:(jc + 1) * JW], ones_bf[:],
                                    t_m[:, jc * JW:(jc + 1) * JW],
                                    start=True, stop=False,
                                    skip_group_check=True)
                        for hc in range(NH):
                            for jc in range(NJ):
                                nc.tensor.matmul(
                                    pss[:, jc * JW:(jc + 1) * JW],
                                    r1s[:, hc, isl],
                                    r2T[:, hc, jc * JW:(jc + 1) * JW],
                                    start=(not use_mask and hc == 0),
                                    stop=(hc == NH - 1),
                                    skip_group_check=True)
                        # PE post-work of the previous chunk goes here, so
                        # the PE never waits on this chunk's softmax.
                        if pending[0] is not None:
                            pending[0]()
                            pending[0] = None
                        # softmax chain (ACT/DVE)
                        nc.scalar.copy(sco[:], pss[:])
                        tneg = smallp.tile([128, 1], dt.float32, tag="tneg")
                        nc.vector.tensor_reduce(
                            tneg[:], sco[:], axis=mybir.AxisListType.X,
                            op=mybir.AluOpType.max, negate=True)
                        te = tep.tile([128, L], dt.bfloat16, tag="te")
                        tz = smallp.tile([128, 1], dt.float32, tag="tz")
                        nc.scalar.activation(
                            te[:], sco[:], mybir.ActivationFunctionType.Exp,
                            bias=tneg[:], scale=1.0, accum_out=tz[:])
                        tzi = smallp.tile([128, 1], dt.float32, tag="tzi")
                        nc.vector.reciprocal(tzi[:], tz[:])

                        def post(b=b, ic=ic, te=te, tzi=tzi):
                            for jc in range(NJ3):
                                pst = ps_sm.tile([128, 128], dt.bfloat16,
                                                 tag="psm")
                                nc.tensor.transpose(
                                    pst[:], te[:, jc * 128:(jc + 1) * 128],
                                    identbf[:])
                                nc.vector.tensor_copy(tET[:, jc, :], pst[:])
                            for dh in range(2):
                                pso = ps_sm.tile([128, 512], dt.float32,
                                                 tag="psm")
                                dsl = slice(dh * 512, (dh + 1) * 512)
                                for jc in range(NJ3):
                                    nc.tensor.matmul(
                                        pso[:], tET[:, jc, :],
                                        t_x2n[:, jc, dsl],
                                        start=(jc == 0), stop=(jc == NJ3 - 1))
                                tout = outp.tile([128, 512], dt.float32,
                                                 tag="tout")
                                nc.vector.tensor_scalar_mul(
                                    tout[:], pso[:], tzi[:])
                                nc.sync.dma_start(
                                    out[b, ic * 128:(ic + 1) * 128, dsl],
                                    tout[:])

                        pending[0] = post
              pending[0]()

            if reps:
                with tc.For_i(0, reps, 1):
                    emit_all_batches()
            else:
                emit_all_batches()

    _legalize_waits(nc, copy.deepcopy(tok))
    return nc


def _prepare_inputs(x1, x2, x2_mask, W, diagonal):
    import ml_dtypes
    x1 = np.ascontiguousarray(x1, dtype=np.float32)
    x2 = np.ascontiguousarray(x2, dtype=np.float32)
    W = np.ascontiguousarray(W, dtype=np.float32)
    diagonal = np.asarray(diagonal, dtype=np.float32)
    mask = np.asarray(x2_mask).astype(np.float32)

    assert np.all(diagonal > 0), "kernel fast path requires diagonal > 0"
    WT = np.ascontiguousarray(W.T, dtype=np.float32).astype(np.float16)
    if np.all(diagonal == 1.0):
        W1T = WT
    else:
        W1T = np.ascontiguousarray((W * diagonal[:, None]).T).astype(np.float16)

    x1T = np.ascontiguousarray(x1.transpose(0, 2, 1)).astype(np.float16)
    x2T = np.ascontiguousarray(x2.transpose(0, 2, 1)).astype(np.float16)
    x2nb = x2.astype(ml_dtypes.bfloat16)
    mrowv = ((1.0 - mask) * NEG)[:, None, :].astype(ml_dtypes.bfloat16)

    in_maps = []
    for c in range(NCORES):
        bs = slice(c * B_LOC, (c + 1) * B_LOC)
        in_maps.append({
            "x1T": x1T[bs],
            "x2T": x2T[bs],
            "WT": WT,
            "W1T": W1T,
            "x2n": x2nb[bs],
            "mrow": mrowv[bs],
        })
    return in_maps


_PROGS = {}


def _get_program(reps=None, use_mask=True):
    key = (reps, use_mask)
    if key not in _PROGS:
        _PROGS[key] = _build_program(reps=reps, use_mask=use_mask)
    return _PROGS[key]


def run(inputs, trace=False, **kw):
    """Run and return (output, BassKernelResults)."""
    from concourse.bass_utils import run_bass_kernel_spmd
    use_mask = not np.all(np.asarray(inputs["x2_mask"]) == 1)
    nc = _get_program(use_mask=use_mask)
    in_maps = _prepare_inputs(**inputs)
    try:
        res = run_bass_kernel_spmd(nc, in_maps, core_ids=list(range(NCORES)),
                                   trace=trace, **kw)
    except Exception:
        # first-compile hiccups have been observed under concurrent load;
        # the NEFF cache makes the retry cheap
        res = run_bass_kernel_spmd(nc, in_maps, core_ids=list(range(NCORES)),
                                   trace=trace, **kw)
    outs = [res.results[c]["out"] for c in range(NCORES)]
    full = np.concatenate(outs, axis=0).astype(np.float32)
    return full, res


def kernel(**inputs) -> np.ndarray:
    out, _ = run(inputs, trace=False)
    return out



# revision 20
# speedup vs baseline: 1.1084x; 1.1084x over previous
"""DiagonalAttention Trainium2 kernel (Bass/Tile), data-parallel over batch.

Reference computation per batch b (L=2048, D=H=1024):
    r1 = relu(x1 @ W.T) * diag        [L, H]   (diag>0 folded into W1)
    r2 = relu(x2 @ W.T)               [L, H]
    s  = r1 @ r2.T + (1-mask)*NEG     [L, L]
    out = softmax(s, -1) @ x2         [L, D]

Device strategy per core (B_LOC=2 batches):
  - host: transpose x1/x2 to [D, L] fp16, W/W1 -> WT fp16, x2 bf16 copy for
    the output matmul, mask row bf16.
  - proj (fp16 matmuls, fp32 psum): rT[h, l] accumulated over d-chunks,
    relu on ScalarE -> fp16.  Input slabs are prefetched one slab ahead
    across phase and batch boundaries (slab pool bufs=2).
  - scores (fp16): psum[i=128, j=2048]; mask row added via K=1 bf16 starter
    matmuls when masking; VectorE row-max reads PSUM directly, ScalarE
    exp(s-max) reads PSUM directly -> bf16 E in SBUF with fused row-sum
    (accum_out).  No PSUM->SBUF score copy.
  - E is transposed on the PE (16 128x128 identity matmuls, each its own
    accumulation group) into ONE packed psum tile [128, 16, 128] bf16
    (2 banks), then a single DVE copy moves it to SBUF.  The transposes
    are emitted AFTER post(i-1) and BEFORE scores(i+1), when te(i) is
    already computed, so the PE never stalls; the copy hides under
    scores(i+1).  (An XBAR DMA-transpose variant was tried and reverted:
    its completion semaphore fires before all tile descriptors land, which
    races with the PE's weight load.)
  - bmm3 = ET.T @ x2_bf16 in psum, scaled by 1/z on VectorE during
    psum->SBUF copy, DMA out on the SP ring.
  - PE stream is software-pipelined one i-chunk: the bmm3 of chunk i is
    emitted after the scores matmuls of chunk i+1, so the PE never stalls
    on the softmax chain.  The post-work of the last chunk of batch b is
    emitted after the first proj2 slab of batch b+1.
  - t_x2n (bmm3 moving operand) is double-buffered across batches so its
    load never serializes against the previous batch's last bmm3.

This container's walrus allows ONE sync wait per instruction; the
legalization pass below splits multi-wait instructions (absorber DMAs on
the same ring for DMAs — HWDGE rings evaluate descriptor waits in FIFO
order — and NoOps on the same engine otherwise).
"""
import copy
import numpy as np

B, L, D, H = 16, 2048, 1024, 1024
NCORES = 8
B_LOC = B // NCORES
NEG = -10000.0

ND = D // 128    # d chunks (contraction of proj)
NH = H // 128    # h chunks
NI = L // 128    # i chunks per batch
SW = 512         # proj slab width (moving dim)
NS = L // SW     # slabs per batch
IPS = SW // 128  # i-chunks per slab
JW = 512         # scores moving width
NJ = L // JW     # j chunks in scores
NJ3 = L // 128   # j chunks in bmm3 (stationary ET tiles)


# ---------------------------------------------------------------------------
# Wait-count legalization


def _make_wait_scratch(nc):
    """Scratch DRAM + token DMA (call inside the TileContext); the token
    gives a fully-lowered physical-AP DMACopy to clone as absorber."""
    from concourse import mybir

    src = nc.dram_tensor("legal_src", [1, 16], mybir.dt.float32, kind="Internal")
    dst = nc.dram_tensor("legal_dst", [1, 16], mybir.dt.float32, kind="Internal")
    tok = nc.sync.dma_start(dst.ap()[0:1, 0:1], src.ap()[0:1, 0:1])
    return tok.ins


def _legalize_waits(nc, template_inst, max_waits=1):
    """Split every instruction with more than max_waits sync waits.

    DMACopy: insert tiny absorber DMAs on the same queue (ring-FIFO makes
    the carried waits gate the real DMA).  Engine instructions: insert
    NoOps on the same engine (engines dispatch strictly in order; a
    waiting NoOp stalls everything behind it).
    """
    from concourse import mybir

    sem = nc.alloc_semaphore("legal_junk")
    junk = mybir.SyncUpdate(
        sync_type="semaphore", id=getattr(sem, "num", None),
        update_mode="sem-add-imm", update_value=16,
        ant_name=getattr(sem, "name", "legal_junk"))
    for fn in nc.m.functions:
        for blk in fn.blocks:
            out = []
            for inst in blk.instructions:
                si = getattr(inst, "sync_info", None)
                if si is None or si.on_wait is None or len(si.on_wait) <= max_waits:
                    out.append(inst)
                    continue
                waits = list(si.on_wait)
                extra, keep = waits[:-max_waits], waits[-max_waits:]
                if isinstance(inst, mybir.InstDMACopy):
                    for w in extra:
                        ab = copy.deepcopy(template_inst)
                        ab.name = nc.get_next_instruction_name()
                        ab.queue = inst.queue
                        ab.sync_info = mybir.SyncInfo(
                            on_wait=[w], on_update=[copy.deepcopy(junk)])
                        out.append(ab)
                else:
                    for w in extra:
                        out.append(mybir.InstNoOp(
                            name=nc.get_next_instruction_name(),
                            engine=inst.engine,
                            sync_info=mybir.SyncInfo(on_wait=[w], on_update=[])))
                inst.sync_info = mybir.SyncInfo(
                    on_wait=keep, on_update=list(si.on_update or []))
                out.append(inst)
            blk.instructions[:] = out


# ---------------------------------------------------------------------------
# Program


def _build_program(reps=None, use_mask=True, legalize=True):
    """Build the program.  reps=k wraps the whole computation in a
    hardware For loop executing it k times — used only for marginal
    HW timing (the ~100 ms axon dispatch swamps a single ~1 ms exec).
    use_mask=False elides the K=1 mask-starter matmuls (the mask input
    must then be all-ones so its additive row is exactly zero)."""
    import concourse.bass as bass
    import concourse.tile as tile
    from concourse import mybir
    from concourse.masks import make_identity

    dt = mybir.dt
    nc = bass.Bass("TRN2", target_bir_lowering=False, debug=False)

    x1T = nc.dram_tensor("x1T", [B_LOC, D, L], dt.float16, kind="ExternalInput").ap()
    x2T = nc.dram_tensor("x2T", [B_LOC, D, L], dt.float16, kind="ExternalInput").ap()
    WT = nc.dram_tensor("WT", [D, H], dt.float16, kind="ExternalInput").ap()
    W1T = nc.dram_tensor("W1T", [D, H], dt.float16, kind="ExternalInput").ap()
    x2n = nc.dram_tensor("x2n", [B_LOC, L, D], dt.bfloat16, kind="ExternalInput").ap()
    mrow = nc.dram_tensor("mrow", [B_LOC, 1, L], dt.bfloat16, kind="ExternalInput").ap()
    out = nc.dram_tensor("out", [B_LOC, L, D], dt.float32, kind="ExternalOutput").ap()

    with tile.TileContext(nc) as tc:
        with (
            tc.tile_pool(name="const", bufs=1) as cpool,
            tc.tile_pool(name="big", bufs=1) as bigp,
            tc.tile_pool(name="x2np", bufs=2) as x2np,
            tc.tile_pool(name="slab", bufs=2) as slabp,
            tc.tile_pool(name="r1pool", bufs=2) as r1p,
            tc.tile_pool(name="tep", bufs=2) as tep,
            tc.tile_pool(name="tetp", bufs=2) as tetp,
            tc.tile_pool(name="small", bufs=2) as smallp,
            tc.tile_pool(name="mp", bufs=2) as mpool,
            tc.tile_pool(name="outp", bufs=4) as outp,
            tc.tile_pool(name="ps_s", bufs=1, space="PSUM") as ps_s,
            tc.tile_pool(name="ps_w", bufs=2, space="PSUM") as psw,
            tc.tile_pool(name="ps_t", bufs=1, space="PSUM") as ps_t,
        ):
            tok = _make_wait_scratch(nc)

            ones_bf = cpool.tile([1, 128], dt.bfloat16, tag="ones_bf")
            nc.vector.memset(ones_bf[:], 1.0)
            identbf = cpool.tile([128, 128], dt.bfloat16, tag="identbf")
            make_identity(nc, identbf[:])

            # resident weights
            wt = bigp.tile([128, ND, H], dt.float16, tag="wt")
            w1t = bigp.tile([128, ND, H], dt.float16, tag="w1t")
            for dc in range(ND):
                nc.sync.dma_start(
                    wt[:, dc, :], WT.rearrange("(c p) h -> p c h", p=128)[:, dc, :])

            # per-batch resident tensor
            r2T = bigp.tile([128, NH, L], dt.float16, tag="r2T")

            def emit_all_batches():
                pending = [None]

                def flush():
                    if pending[0] is not None:
                        pending[0]()
                        pending[0] = None

                # slab prefetch: one slab ahead across phases and batches
                slab_specs = []
                for b in range(B_LOC):
                    for s in range(NS):
                        slab_specs.append((x2T, b, s))
                    for s in range(NS):
                        slab_specs.append((x1T, b, s))
                slab_tiles = {}

                def issue_slab(k):
                    if k >= len(slab_specs):
                        return
                    src, b, s = slab_specs[k]
                    xs = slabp.tile([128, ND, SW], dt.float16, tag="xslab")
                    for dc in range(ND):
                        nc.sync.dma_start(
                            xs[:, dc, :],
                            src[b].rearrange("(c p) l -> p c l", p=128)[
                                :, dc, s * SW:(s + 1) * SW])
                    slab_tiles[k] = xs

                def get_slab(k):
                    xs = slab_tiles.pop(k)
                    issue_slab(k + 1)
                    return xs

                issue_slab(0)
                for b in range(B_LOC):
                    t_x2n = x2np.tile([128, NJ3, D], dt.bfloat16, tag="t_x2n")
                    if use_mask:
                        t_m = mpool.tile([1, L], dt.bfloat16, tag="t_m")

                    # ---- proj2: r2T = relu(W @ x2T) ----
                    for s in range(NS):
                        xs = get_slab(2 * NS * b + s)
                        for hc in range(NH):
                            psp = psw.tile([128, SW], dt.float32, tag="psw")
                            for dc in range(ND):
                                nc.tensor.matmul(
                                    psp[:], wt[:, dc, hc * 128:(hc + 1) * 128],
                                    xs[:, dc, :],
                                    start=(dc == 0), stop=(dc == ND - 1))
                            nc.scalar.activation(
                                r2T[:, hc, s * SW:(s + 1) * SW], psp[:],
                                mybir.ActivationFunctionType.Relu)
                        if s == 0:
                            # previous batch's last post hides behind proj2
                            flush()
                            if b == 0:
                                for dc in range(ND):
                                    nc.sync.dma_start(
                                        w1t[:, dc, :],
                                        W1T.rearrange("(c p) h -> p c h",
                                                      p=128)[:, dc, :])
                        if s == 1:
                            for jc in range(NJ3):
                                nc.sync.dma_start(
                                    t_x2n[:, jc, :],
                                    x2n[b].rearrange("(c p) d -> p c d",
                                                     p=128)[:, jc, :])
                            if use_mask:
                                nc.sync.dma_start(t_m[:], mrow[b])

                    # ---- proj1 + attention, slab by slab ----
                    # proj1 state per slab: hc groups may be hoisted early to
                    # fill the cold-pipe bubble at the batch's first chunk.
                    p1_state = {}

                    def emit_proj1_hc(s, hc):
                        xs, r1s, done = p1_state[s]
                        if hc in done:
                            return
                        done.add(hc)
                        psp = psw.tile([128, SW], dt.float32, tag="psw")
                        for dc in range(ND):
                            nc.tensor.matmul(
                                psp[:], w1t[:, dc, hc * 128:(hc + 1) * 128],
                                xs[:, dc, :],
                                start=(dc == 0), stop=(dc == ND - 1))
                        nc.scalar.activation(
                            r1s[:, hc, :], psp[:],
                            mybir.ActivationFunctionType.Relu)

                    def open_p1_slab(s):
                        if s in p1_state:
                            return
                        xs = get_slab(2 * NS * b + NS + s)
                        r1s = r1p.tile([128, NH, SW], dt.float16,
                                       tag="r1slab")
                        p1_state[s] = (xs, r1s, set())

                    for s in range(NS):
                        open_p1_slab(s)
                        for hc in range(NH):
                            emit_proj1_hc(s, hc)
                        r1s = p1_state[s][1]

                        for il in range(IPS):
                            ic = s * IPS + il
                            isl = slice(il * 128, (il + 1) * 128)
                            pss = ps_s.tile([128, L], dt.float32, tag="pss")
                            if use_mask:
                                for jc in range(NJ):
                                    nc.tensor.matmul(
                                        pss[:, jc * JW:(jc + 1) * JW],
                                        ones_bf[:],
                                        t_m[:, jc * JW:(jc + 1) * JW],
                                        start=True, stop=False,
                                        skip_group_check=True)
                            for hc in range(NH):
                                for jc in range(NJ):
                                    nc.tensor.matmul(
                                        pss[:, jc * JW:(jc + 1) * JW],
                                        r1s[:, hc, isl],
                                        r2T[:, hc, jc * JW:(jc + 1) * JW],
                                        start=(not use_mask and hc == 0),
                                        stop=(hc == NH - 1),
                                        skip_group_check=True)
                            # PE post-work of the previous chunk goes here,
                            # so the PE never waits on this chunk's softmax.
                            cold = pending[0] is None
                            flush()
                            if cold and s + 1 < NS:
                                # cold pipe: nothing covers this chunk's
                                # softmax chain before the pss WAR and the
                                # transposes — fill the bubble with the next
                                # slab's first proj1 groups.
                                open_p1_slab(s + 1)
                                for hc in range(3):
                                    emit_proj1_hc(s + 1, hc)
                            # softmax chain (DVE/ACT) straight from PSUM
                            tneg = smallp.tile([128, 1], dt.float32, tag="tneg")
                            nc.vector.tensor_reduce(
                                tneg[:], pss[:], axis=mybir.AxisListType.X,
                                op=mybir.AluOpType.max, negate=True)
                            te = tep.tile([128, L], dt.bfloat16, tag="te")
                            tz = smallp.tile([128, 1], dt.float32, tag="tz")
                            nc.scalar.activation(
                                te[:], pss[:],
                                mybir.ActivationFunctionType.Exp,
                                bias=tneg[:], scale=1.0, accum_out=tz[:])
                            tzi = smallp.tile([128, 1], dt.float32, tag="tzi")
                            nc.vector.reciprocal(tzi[:], tz[:])
                            # PE transposes of te into one packed psum tile
                            # (2 banks).  te(i) is ready by now (post(i-1),
                            # or the hoisted proj1 groups on a cold pipe,
                            # covered the softmax chain), so the PE does not
                            # stall, and the single DVE copy hides under
                            # scores(i+1).
                            ptr = ps_t.tile([128, NJ3, 128], dt.bfloat16,
                                            tag="ptr")
                            for jc in range(NJ3):
                                nc.tensor.matmul(
                                    ptr[:, jc, :],
                                    te[:, jc * 128:(jc + 1) * 128],
                                    identbf[:], is_transpose=True,
                                    start=True, stop=True,
                                    skip_group_check=True)
                            tET = tetp.tile([128, NJ3, 128], dt.bfloat16,
                                            tag="tET")
                            nc.vector.tensor_copy(tET[:], ptr[:])

                            def post(b=b, ic=ic, tET=tET, tzi=tzi,
                                     t_x2n=t_x2n):
                                for dh in range(2):
                                    pso = psw.tile([128, 512], dt.float32,
                                                   tag="psw")
                                    dsl = slice(dh * 512, (dh + 1) * 512)
                                    for jc in range(NJ3):
                                        nc.tensor.matmul(
                                            pso[:], tET[:, jc, :],
                                            t_x2n[:, jc, dsl],
                                            start=(jc == 0),
                                            stop=(jc == NJ3 - 1))
                                    tout = outp.tile([128, 512], dt.float32,
                                                     tag="tout")
                                    nc.vector.tensor_scalar_mul(
                                        tout[:], pso[:], tzi[:])
                                    nc.sync.dma_start(
                                        out[b, ic * 128:(ic + 1) * 128, dsl],
                                        tout[:])

                            pending[0] = post
                flush()

            if reps:
                with tc.For_i(0, reps, 1):
                    emit_all_batches()
            else:
                emit_all_batches()

    if legalize:
        _legalize_waits(nc, copy.deepcopy(tok))
    return nc


def _prepare_inputs(x1, x2, x2_mask, W, diagonal):
    import ml_dtypes
    x1 = np.ascontiguousarray(x1, dtype=np.float32)
    x2 = np.ascontiguousarray(x2, dtype=np.float32)
    W = np.ascontiguousarray(W, dtype=np.float32)
    diagonal = np.asarray(diagonal, dtype=np.float32)
    mask = np.asarray(x2_mask).astype(np.float32)

    assert np.all(diagonal > 0), "kernel fast path requires diagonal > 0"
    WT = np.ascontiguousarray(W.T, dtype=np.float32).astype(np.float16)
    if np.all(diagonal == 1.0):
        W1T = WT
    else:
        W1T = np.ascontiguousarray((W * diagonal[:, None]).T).astype(np.float16)

    x1T = np.ascontiguousarray(x1.transpose(0, 2, 1)).astype(np.float16)
    x2T = np.ascontiguousarray(x2.transpose(0, 2, 1)).astype(np.float16)
    x2nb = x2.astype(ml_dtypes.bfloat16)
    mrowv = ((1.0 - mask) * NEG)[:, None, :].astype(ml_dtypes.bfloat16)

    in_maps = []
    for c in range(NCORES):
        bs = slice(c * B_LOC, (c + 1) * B_LOC)
        in_maps.append({
            "x1T": x1T[bs],
            "x2T": x2T[bs],
            "WT": WT,
            "W1T": W1T,
            "x2n": x2nb[bs],
            "mrow": mrowv[bs],
        })
    return in_maps


_PROGS = {}


def _get_program(reps=None, use_mask=True):
    key = (reps, use_mask)
    if key not in _PROGS:
        _PROGS[key] = _build_program(reps=reps, use_mask=use_mask)
    return _PROGS[key]


def run(inputs, trace=False, **kw):
    """Run and return (output, BassKernelResults)."""
    from concourse.bass_utils import run_bass_kernel_spmd
    use_mask = not np.all(np.asarray(inputs["x2_mask"]) == 1)
    nc = _get_program(use_mask=use_mask)
    in_maps = _prepare_inputs(**inputs)
    try:
        res = run_bass_kernel_spmd(nc, in_maps, core_ids=list(range(NCORES)),
                                   trace=trace, **kw)
    except Exception:
        # first-compile hiccups have been observed under concurrent load;
        # the NEFF cache makes the retry cheap
        res = run_bass_kernel_spmd(nc, in_maps, core_ids=list(range(NCORES)),
                                   trace=trace, **kw)
    outs = [res.results[c]["out"] for c in range(NCORES)]
    full = np.concatenate(outs, axis=0).astype(np.float32)
    return full, res


def kernel(**inputs) -> np.ndarray:
    out, _ = run(inputs, trace=False)
    return out
